# revision 11
# baseline (speedup 1.0000x reference)
"""Trainium2 Bass kernel for nn_DSNN: 3-layer spiking NN, T=64 steps.

Strategy (per core, batch-sharded 2048 -> 8 x 256):
- Feature-major on-device layout: state tiles are [128 partitions(f), cols] with
  f = chunk*128 + p, batch along free dim.
- Layer 0: input is constant across time -> h0 = x @ w0 computed ONCE (exact
  fp32 matmuls), the time loop only runs the membrane decay/spike/reset.
- Layer 1 matmul per step uses sigma = sign(mem - 1) in {-1,0,1} (computed on
  ScalarE) instead of 0/1 spikes: h1 = sigma @ (w1/2) + colsum(w1)/2, with the
  constant column-sum added via rank-1 matmuls. This keeps VectorE off the
  spike computation.
- Membrane reset in ONE VectorE instruction: mem' = (nm is_le 1.0) mult nm.
- Layer 2 (512->6) is reconstructed on the host from the exact fp8 0/1 spike
  records of layer 1 (0.06% of the FLOPs, removes all layer-2 device work).
- Records DMA'd per step as two wide 128-partition transfers: membranes fp32
  (1 MiB) + spikes fp8 (256 KiB).
"""
import sys, time
sys.path.insert(0, "/opt/trn_rl_repo")
import numpy as np
import ml_dtypes
from contextlib import ExitStack

import concourse.bass as bass
import concourse.bacc as bacc
import concourse.tile as tile
from concourse import mybir

F32 = mybir.dt.float32
F32R = mybir.dt.float32r
BF16 = mybir.dt.bfloat16
FP8 = mybir.dt.float8e4
AF = mybir.ActivationFunctionType
OP = mybir.AluOpType

ALPHA, BETA, THRESH = 0.95, 0.9, 1.0
T = 64
B, DIN, H, OUT = 2048, 128, 512, 6
NCORES = 8
BL = B // NCORES          # 256 batch rows per core
NC_F = H // 128           # 4 feature chunks of the 512-wide hidden layers

# matmul mode for the recurrent layer-1 matmul:
#  "f32r"    : 16 MMs/step, fp32 data in reduced-precision fast mode (~1e-4)
#  "f32r_x2" : 32 MMs/step, hi/lo split pre-truncated to 12 mantissa bits
#  "bf16x2"  : 32 MMs/step, bf16 hi/lo split
#  "f32"     : 16 MMs/step at 4 cycles/row, exact fp32 (slow)
MM_MODE = "f32r_x2"


def _trunc_mantissa(x, bits):
    keep = np.uint32(0xFFFFFFFF) << np.uint32(23 - bits)
    return (x.view(np.uint32) & keep).view(np.float32)


def build_bass(n_steps=T, mm_mode=MM_MODE):
    nc = bacc.Bacc()
    two_pass = mm_mode in ("f32r_x2", "bf16x2")
    mmdt = {"f32r": F32R, "f32r_x2": F32R, "bf16x2": BF16, "f32": F32}[mm_mode]
    wdt = mmdt
    nw = 2 if two_pass else 1

    # ---- DRAM I/O ----
    xT_in = nc.dram_tensor("xT", [DIN, BL], F32, kind="ExternalInput")
    w0_in = nc.dram_tensor("w0m", [DIN, H], F32, kind="ExternalInput")
    bias0_in = nc.dram_tensor("bias0", [128, NC_F], F32, kind="ExternalInput")
    # w1 tiles: k-chunk k occupies cols [k*H, (k+1)*H); values w1/2 (possibly hi/lo)
    w1_in = nc.dram_tensor("w1t", [nw, 128, NC_F * H], wdt, kind="ExternalInput")
    c1_in = nc.dram_tensor("c1t", [1, 2 * H], wdt, kind="ExternalInput")  # hi|lo rank-1
    ones_in = nc.dram_tensor("ones", [1, BL], wdt, kind="ExternalInput")
    mem_out = nc.dram_tensor("mem01_rec", [n_steps, 128, 2 * NC_F * BL], F32, kind="ExternalOutput")
    spk_out = nc.dram_tensor("spk01_rec", [n_steps, 128, 2 * NC_F * BL], FP8, kind="ExternalOutput")

    with ExitStack() as ctx:
        tc = ctx.enter_context(tile.TileContext(nc))
        consts = ctx.enter_context(tc.tile_pool(name="consts", bufs=1))
        statep = ctx.enter_context(tc.tile_pool(name="state", bufs=4))
        synp = ctx.enter_context(tc.tile_pool(name="syn", bufs=2))
        tmpp = ctx.enter_context(tc.tile_pool(name="tmp", bufs=3))
        spkp = ctx.enter_context(tc.tile_pool(name="spk", bufs=4))
        psp = ctx.enter_context(tc.tile_pool(name="ps", bufs=2, space="PSUM"))
        psp0 = ctx.enter_context(tc.tile_pool(name="ps0", bufs=1, space="PSUM"))

        # ---- load constants ----
        xT = consts.tile([DIN, BL], F32)
        w0t = consts.tile([DIN, H], F32)
        bias0 = consts.tile([128, NC_F], F32)
        w1t = consts.tile([128, nw * NC_F * H], wdt)
        c1t = consts.tile([1, 2 * H], wdt)
        ones = consts.tile([1, BL], wdt)
        nc.sync.dma_start(xT[:], xT_in[:])
        nc.sync.dma_start(w0t[:], w0_in[:])
        nc.sync.dma_start(bias0[:], bias0_in[:])
        for i in range(nw):
            nc.sync.dma_start(w1t[:, i * NC_F * H:(i + 1) * NC_F * H], w1_in[i])
        nc.sync.dma_start(c1t[:], c1_in[:])
        nc.sync.dma_start(ones[:], ones_in[:])
        neg_thr = consts.tile([128, 1], F32)
        half = consts.tile([128, 1], F32)
        nc.vector.memset(neg_thr[:], -THRESH)
        nc.vector.memset(half[:], 0.5)

        # ---- one-time h0 = xT.T-free matmul + bias (exact fp32) ----
        h0 = consts.tile([128, NC_F * BL], F32)
        ps0 = psp0.tile([128, NC_F * BL], F32)
        for j in range(NC_F):
            nc.tensor.matmul(ps0[:, j * BL:(j + 1) * BL],
                             w0t[:, j * 128:(j + 1) * 128], xT[:],
                             start=True, stop=True)
        for j in range(NC_F):
            nc.scalar.activation(h0[:, j * BL:(j + 1) * BL],
                                 ps0[:, j * BL:(j + 1) * BL],
                                 AF.Identity, bias=bias0[:, j:j + 1], scale=1.0)

        # ---- initial states ----
        mem01 = statep.tile([128, 2 * NC_F * BL], F32)   # [L0 | L1] halves
        syn1 = synp.tile([128, NC_F * BL], F32)
        nc.vector.memset(mem01[:], 0.0)
        nc.vector.memset(syn1[:], 0.0)

        HW_ = NC_F * BL  # 1024 columns per layer half

        # ---- time loop ----
        for t in range(n_steps):
            # L0 membrane update (VectorE), nm0 = beta*mem0 + h0
            nm0 = tmpp.tile([128, HW_], F32, tag="nm0")
            nc.vector.scalar_tensor_tensor(nm0[:], mem01[:, :HW_], BETA, h0[:],
                                           op0=OP.mult, op1=OP.add)
            # sigma0 = sign(nm0 - 1) on ScalarE
            sg0 = tmpp.tile([128, HW_], mmdt, tag="sg0")
            nc.scalar.activation(sg0[:], nm0[:], AF.Sign, bias=neg_thr[:], scale=1.0)

            spk8 = spkp.tile([128, 2 * HW_], FP8, tag="spk8")
            # spike record L0 = 0.5*sigma0 + 0.5 (ScalarE, fp8 out)
            nc.scalar.activation(spk8[:, :HW_], sg0[:], AF.Identity,
                                 bias=half[:], scale=0.5)

            mem01_new = statep.tile([128, 2 * NC_F * BL], F32, tag="state")
            # L0 reset on GpSimd: notspk = (nm0 <= 1); mem0' = nm0 * notspk
            nsp0 = tmpp.tile([128, HW_], F32, tag="nsp0")
            nc.gpsimd.tensor_scalar(nsp0[:], nm0[:], THRESH, None, op0=OP.is_le)
            nc.gpsimd.tensor_tensor(mem01_new[:, :HW_], nm0[:], nsp0[:], op=OP.mult)

            # L1 matmul: psum1 = sigma0 @ (w1/2) + c1 (rank-1)
            ps1 = psp.tile([128, NC_F * BL], F32, tag="ps1")
            for j in range(NC_F):
                out = ps1[:, j * BL:(j + 1) * BL]
                nc.tensor.matmul(out, (c1t[0:1, j * 128:(j + 1) * 128]),
                                 (ones[:]), start=True, stop=False)
                nc.tensor.matmul(out, (c1t[0:1, H + j * 128:H + (j + 1) * 128]),
                                 (ones[:]), start=False, stop=False)
                for i in range(nw):
                    for k in range(NC_F):
                        lhs = w1t[:, i * NC_F * H + k * H + j * 128:
                                  i * NC_F * H + k * H + (j + 1) * 128]
                        nc.tensor.matmul(out, (lhs),
                                         (sg0[:, k * BL:(k + 1) * BL]),
                                         start=False,
                                         stop=(i == nw - 1 and k == NC_F - 1))

            # L1 syn + membrane (VectorE)
            syn1_new = synp.tile([128, HW_], F32, tag="syn")
            nc.vector.scalar_tensor_tensor(syn1_new[:], syn1[:], ALPHA, ps1[:],
                                           op0=OP.mult, op1=OP.add)
            nm1 = tmpp.tile([128, HW_], F32, tag="nm1")
            nc.vector.scalar_tensor_tensor(nm1[:], mem01[:, HW_:], BETA, syn1_new[:],
                                           op0=OP.mult, op1=OP.add)
            # spike record L1 via ScalarE: sigma1 then 0.5*sigma1+0.5 -> fp8
            sg1 = tmpp.tile([128, HW_], F32, tag="sg1")
            nc.scalar.activation(sg1[:], nm1[:], AF.Sign, bias=neg_thr[:], scale=1.0)
            nc.scalar.activation(spk8[:, HW_:], sg1[:], AF.Identity,
                                 bias=half[:], scale=0.5)
            # L1 reset (VectorE, one STT)
            nc.vector.scalar_tensor_tensor(mem01_new[:, HW_:], nm1[:], THRESH, nm1[:],
                                           op0=OP.is_le, op1=OP.mult)

            # record DMAs (HWDGE)
            nc.sync.dma_start(mem_out[t], mem01_new[:])
            nc.sync.dma_start(spk_out[t], spk8[:])

            mem01 = mem01_new
            syn1 = syn1_new

    nc.compile()
    return nc


def make_inputs(inputs, w0, w1):
    """Host-side prep: per-core input dicts."""
    w0 = np.asarray(w0, np.float32)
    w1 = np.asarray(w1, np.float32)
    x = np.asarray(inputs, np.float32)

    w0m = np.ascontiguousarray(w0[:DIN])                       # [128, 512]
    bias0 = np.ascontiguousarray(
        (0.5 * w0[DIN]).reshape(NC_F, 128).T)                  # [128, 4]
    w1half = (w1 * 0.5).astype(np.float32)
    c1 = (w1.astype(np.float64).sum(axis=0) * 0.5).astype(np.float32)

    if MM_MODE == "bf16x2":
        hi = w1half.astype(ml_dtypes.bfloat16)
        lo = (w1half - hi.astype(np.float32)).astype(ml_dtypes.bfloat16)
        parts = [hi, lo]
        wdt = ml_dtypes.bfloat16
        c1hi = c1.astype(ml_dtypes.bfloat16)
        c1lo = (c1 - c1hi.astype(np.float32)).astype(ml_dtypes.bfloat16)
    elif MM_MODE == "f32r_x2":
        # fp32r keeps exactly 11 mantissa bits (HW-probed) -> 11+11 split
        hi = _trunc_mantissa(w1half, 11)
        lo = _trunc_mantissa((w1half - hi).astype(np.float32), 11)
        parts = [hi, lo]
        wdt = np.float32
        c1hi = _trunc_mantissa(c1, 11)
        c1lo = _trunc_mantissa((c1 - c1hi).astype(np.float32), 11)
    else:
        parts = [w1half]
        wdt = np.float32
        c1hi = _trunc_mantissa(c1, 12)
        c1lo = (c1 - c1hi).astype(np.float32)
    c1t = np.concatenate([np.asarray(c1hi), np.asarray(c1lo)])[None, :].astype(wdt)
    # each part -> [128, 4*512]: k-chunk k at cols [k*512,(k+1)*512)
    w1t = np.stack([
        np.concatenate([p[k * 128:(k + 1) * 128] for k in range(NC_F)], axis=1)
        for p in parts]).astype(wdt)                           # [nw, 128, 2048]

    in_maps = []
    for c in range(NCORES):
        xT = np.ascontiguousarray(x[c * BL:(c + 1) * BL].T)    # [128, 256]
        in_maps.append({"xT": xT, "w0m": w0m, "bias0": bias0,
                        "w1t": w1t, "c1t": c1t,
                        "ones": np.ones((1, BL), wdt)})
    return in_maps


_CACHE = {}


def _get_nc():
    key = (T, MM_MODE)
    if key not in _CACHE:
        _CACHE[key] = build_bass(T, MM_MODE)
    return _CACHE[key]


def run_device(in_maps):
    from concourse.bass_utils import run_bass_kernel_spmd
    nc = _get_nc()
    res = run_bass_kernel_spmd(nc, in_maps, list(range(NCORES)))
    return res.results


def postprocess(core_results, w2):
    """Assemble full outputs + host-side layer 2."""
    w2 = np.asarray(w2, np.float32)
    a, b_, th = np.float32(ALPHA), np.float32(BETA), np.float32(THRESH)

    mem0 = np.empty((T, B, H), np.float32)
    mem1 = np.empty((T, B, H), np.float32)
    spk0 = np.empty((T, B, H), np.float32)
    spk1 = np.empty((T, B, H), np.float32)
    for c, r in enumerate(core_results):
        m = r["mem01_rec"]                                   # [T,128,2048] f32
        s = np.asarray(r["spk01_rec"].view(ml_dtypes.float8_e4m3)
                       if r["spk01_rec"].dtype.itemsize == 1 and r["spk01_rec"].dtype != ml_dtypes.float8_e4m3
                       else r["spk01_rec"]).astype(np.float32)
        sl = slice(c * BL, (c + 1) * BL)
        # cols [:1024] = L0, [1024:] = L1 ; [128p, 4chunk, 256b] -> [b, f=c4*128+p]
        def unpack(arr, half):
            v = arr[:, :, half * 1024:(half + 1) * 1024].reshape(T, 128, NC_F, BL)
            return np.ascontiguousarray(v.transpose(0, 3, 2, 1)).reshape(T, BL, H)
        mem0[:, sl] = unpack(m, 0)
        mem1[:, sl] = unpack(m, 1)
        spk0[:, sl] = unpack(s, 0)
        spk1[:, sl] = unpack(s, 1)

    # host layer 2 (exact 0/1 spikes -> fp32 BLAS matmul, matches ref order)
    h2 = spk1.reshape(T * B, H) @ w2                          # [T*B, 6] f32
    h2 = h2.reshape(T, B, OUT).astype(np.float32)
    ns = np.zeros((B, OUT), np.float32)
    nm = np.zeros((B, OUT), np.float32)
    mem2 = np.empty((T, B, OUT), np.float32)
    spk2 = np.empty((T, B, OUT), np.float32)
    for t in range(T):
        ns = a * ns + h2[t]
        nm = b_ * nm + ns
        mem2[t] = nm
        spk2[t] = (nm - th > 0).astype(np.float32)
    return (mem2[-1].copy(),
            (mem0, mem1, mem2),
            (spk0, spk1, spk2))


def kernel(inputs, w0, w1, w2):
    in_maps = make_inputs(inputs, w0, w1)
    core_results = run_device(in_maps)
    return postprocess(core_results, w2)
